# revision 6
# baseline (speedup 1.0000x reference)
"""Trainium2 Bass kernel for a 2-layer causal transformer decoder with a
ragged repeat-expand prologue.

Distribution: DP=2 over batch x TP=4 over heads / FFN hidden dim
(cores 0-3 -> batch 0, cores 4-7 -> batch 1).  Within a TP group the
residual stream is sequence-sharded; each AllGather / ReduceScatter is
split into NCHUNK chunks that pipeline with the surrounding matmuls so
collective latency is hidden.  Row ownership is strided by row-tile so
that a ReduceScatter chunk's output lands exactly on its owner.

v2 layout notes:
 - residual x kept in fp16; reps/resid pre-cast on the host.
 - h^T (channel-major activations) produced by XBAR DMA-transposes from
   the AllGather output (no DVE transposes, no block-permute DMAs).
 - ht is one persistent [P, KTD, S] tile; per-chunk region deps give
   cross-phase pipelining.
 - RoPE rotation via a sign-folded permutation matmul (no partition
   shift DMAs).
 - flash attention processes head pairs per k-tile with one [P, 2*QC]
   exp on the scalar engine.
"""

import numpy as np

import concourse.bass as bass
import concourse.mybir as mybir
import concourse.tile as tile
from concourse import bacc
from concourse import bass_utils

P = 128
EPS = 1e-5
NCORES = 8
TP = 4
DP = 2
GROUPS = [[0, 1, 2, 3], [4, 5, 6, 7]]

F16 = mybir.dt.float16
F32 = mybir.dt.float32
I32 = mybir.dt.int32
AF = mybir.ActivationFunctionType
ALU = mybir.AluOpType

FULL_CFG = dict(B=2, S=2048, D=1024, K=256, H=16, HD=64, F=4096, L=2)


def _derive(cfg):
    d = dict(cfg)
    d["LH"] = cfg["H"] // TP               # local heads
    d["OC"] = d["LH"] * cfg["HD"]          # local attention channels
    assert d["OC"] % P == 0
    d["NOPT"] = d["OC"] // P               # q/k/v/o channel partition-tiles
    d["HPP"] = P // cfg["HD"]              # heads per partition-tile
    d["OWN"] = cfg["S"] // TP              # rows owned by this core
    assert d["OWN"] % P == 0
    d["NRT"] = d["OWN"] // P               # own row tiles
    d["NRA"] = cfg["S"] // P               # all row tiles
    d["KTD"] = cfg["D"] // P               # contraction tiles over D
    d["FL"] = cfg["F"] // TP               # local FFN width
    d["KTF"] = d["FL"] // P                # contraction tiles over FL
    d["NCHUNK"] = d["NRT"]                 # collective chunks (R = 1)
    d["R"] = d["NRT"] // d["NCHUNK"]       # own row-tiles per chunk
    d["CW"] = TP * d["R"] * P              # global rows per chunk
    d["QC"] = d["CW"]                      # attention q chunk == chunk width
    assert d["QC"] <= 512
    d["NQC"] = cfg["S"] // d["QC"]
    assert d["NQC"] == d["NCHUNK"]
    assert cfg["HD"] == 64
    return d


def build_nc(cfg):
    """Builds the SPMD Bass program (identical on all 8 cores)."""
    c = _derive(cfg)
    B, S, D, K, H, HD, F, L = (c[k] for k in ("B", "S", "D", "K", "H", "HD", "F", "L"))
    LH, OC, NOPT, HPP = c["LH"], c["OC"], c["NOPT"], c["HPP"]
    OWN, NRT, NRA, KTD = c["OWN"], c["NRT"], c["NRA"], c["KTD"]
    FL, KTF, QC, NQC = c["FL"], c["KTF"], c["QC"], c["NQC"]
    NCHUNK, R, CW = c["NCHUNK"], c["R"], c["CW"]
    RPC = CW // P                          # global row-tiles per chunk
    ISCALE = float(HD) ** -0.5

    nc = bacc.Bacc("TRN2", target_bir_lowering=False, debug=False,
                   num_devices=NCORES)

    # ---- per-core external inputs (host pre-sharded / pre-laid-out) ----
    reps_d = nc.dram_tensor("reps", [K + 1, D], F16, kind="ExternalInput")
    segq_d = nc.dram_tensor("segq", [OWN, 1], I32, kind="ExternalInput")
    resid_d = nc.dram_tensor("resid", [OWN, D], F16, kind="ExternalInput")
    cosb_d = nc.dram_tensor("cosb", [P, S], F16, kind="ExternalInput")
    sinb_d = nc.dram_tensor("sinb", [P, S], F16, kind="ExternalInput")
    rott_d = nc.dram_tensor("rott", [P, P], F16, kind="ExternalInput")
    fnormb_d = nc.dram_tensor("fnormb", [P, D], F16, kind="ExternalInput")
    masks_d = nc.dram_tensor("masks", [P, RPC * 2 * QC], F16, kind="ExternalInput")
    wq_d = nc.dram_tensor("wq", [L, P, KTD * OC], F16, kind="ExternalInput")
    wk_d = nc.dram_tensor("wk", [L, P, KTD * OC], F16, kind="ExternalInput")
    wv_d = nc.dram_tensor("wv", [L, P, KTD * OC], F16, kind="ExternalInput")
    wo_d = nc.dram_tensor("wo", [L, P, NOPT * D], F16, kind="ExternalInput")
    wg_d = nc.dram_tensor("wg", [L, P, KTD * FL], F16, kind="ExternalInput")
    wu_d = nc.dram_tensor("wu", [L, P, KTD * FL], F16, kind="ExternalInput")
    wd_d = nc.dram_tensor("wd", [L, P, KTF * D], F16, kind="ExternalInput")
    out_d = nc.dram_tensor("out", [OWN, D], F32, kind="ExternalOutput")

    d_chunks = [(s, min(512, D - s)) for s in range(0, D, 512)]

    with tile.TileContext(nc) as tc:
        with tc.tile_pool(name="const", bufs=1) as cpool, \
             tc.tile_pool(name="xres", bufs=1) as xpool, \
             tc.tile_pool(name="ht", bufs=1) as hpool, \
             tc.tile_pool(name="big", bufs=1) as bigpool, \
             tc.tile_pool(name="vv", bufs=1) as vpool, \
             tc.tile_pool(name="wts", bufs=1) as wpool, \
             tc.tile_pool(name="scr", bufs=2) as spool, \
             tc.tile_pool(name="psum", bufs=1, space="PSUM") as ppool, \
             tc.tile_pool(name="dram", bufs=1, space="DRAM") as dpool:

            # ---- ragged expand: x_own = reps[seg] + resid (local order) ----
            segq = spool.tile([P, NRT], I32, name="segq", tag="segq", bufs=1)
            nc.sync.dma_start(
                segq[:], segq_d.rearrange("(i p) o -> p (i o)", p=P))
            x_own = []
            for i in range(NRT):
                xt = xpool.tile([P, D], F16, name=f"x{i}", tag=f"x{i}")
                nc.gpsimd.indirect_dma_start(
                    out=xt[:], out_offset=None, in_=reps_d[:],
                    in_offset=bass.IndirectOffsetOnAxis(ap=segq[:, i:i + 1],
                                                        axis=0))
                res = spool.tile([P, D], F16, name="res", tag="ow", bufs=2)
                nc.sync.dma_start(res[:], resid_d[i * P:(i + 1) * P, :])
                nc.vector.tensor_add(xt[:], xt[:], res[:])
                x_own.append(xt)

            # ---- constants (scalar queue keeps sync free for the expand) ----
            ones64 = cpool.tile([1, 64], F32, name="ones64")
            nc.vector.memset(ones64[:], 1.0)
            zb = cpool.tile([P, 1], F32, name="zb")
            nc.vector.memset(zb[:], 0.0)
            eb = cpool.tile([P, 1], F32, name="eb")
            nc.vector.memset(eb[:], EPS)
            cosb = cpool.tile([P, S], F16, name="cosb")
            nc.scalar.dma_start(cosb[:], cosb_d[:])
            sinb = cpool.tile([P, S], F16, name="sinb")
            nc.scalar.dma_start(sinb[:], sinb_d[:])
            rott = cpool.tile([P, P], F16, name="rott")
            nc.scalar.dma_start(rott[:], rott_d[:])
            fnormb = cpool.tile([P, D], F16, name="fnormb")
            nc.scalar.dma_start(fnormb[:], fnormb_d[:])
            maskt = cpool.tile([P, RPC * 2 * QC], F16, name="maskt")
            nc.scalar.dma_start(maskt[:], masks_d[:])

            # h^T: persistent channel-major activations [P, KTD, S]
            ht = hpool.tile([P, KTD, S], F16, name="ht", tag="ht")

            def rms_inv(src_ap, scratch_ap):
                ss = spool.tile([P, 1], F32, name="ss", tag="ss", bufs=2)
                nc.scalar.activation(scratch_ap, src_ap, AF.Square,
                                     bias=zb[:, :1], accum_out=ss[:])
                st = spool.tile([P, 1], F32, name="st", tag="st", bufs=2)
                nc.scalar.activation(st[:], ss[:], AF.Sqrt, scale=1.0 / D,
                                     bias=eb[:, :1])
                inv = spool.tile([P, 1], F32, name="inv", tag="inv", bufs=2)
                nc.vector.reciprocal_approx_fast(inv[:], st[:])
                return inv

            def ln_ag(phase, cc):
                """LN chunk cc of x_own, AllGather, XBAR-transpose into ht."""
                agin = dpool.tile([R * P, D], F16, name=f"agin_{phase}_{cc}",
                                  tag="agin", bufs=2 * NCHUNK)
                for j in range(R):
                    i = cc * R + j
                    h = spool.tile([P, D], F16, name="h", tag="h", bufs=2)
                    inv = rms_inv(x_own[i][:], h[:])
                    nc.vector.tensor_scalar_mul(h[:], x_own[i][:], inv[:, :1])
                    nc.sync.dma_start(agin[j * P:(j + 1) * P, :], h[:])
                agout = dpool.tile([TP, R * P, D], F16,
                                   name=f"agout_{phase}_{cc}", tag="agout",
                                   bufs=2 * NCHUNK)
                nc.gpsimd.collective_compute(
                    "AllGather", ALU.bypass, replica_groups=GROUPS,
                    ins=[agin[:]], outs=[agout[:]])
                for kt in range(KTD):
                    nc.sync.dma_start_transpose(
                        ht[:, kt, cc * CW:(cc + 1) * CW],
                        agout[:, :, kt * P:(kt + 1) * P].rearrange(
                            "r w c -> (r w) c"))

            def rs_chunk_add(rsin, phase, cc):
                """ReduceScatter chunk cc and add into own rows."""
                rsout = dpool.tile([R * P, D], F16, name=f"rso_{phase}_{cc}",
                                   tag="rsout", bufs=2 * NCHUNK)
                nc.gpsimd.collective_compute(
                    "ReduceScatter", ALU.add, replica_groups=GROUPS,
                    ins=[rsin[:]], outs=[rsout[:]])
                for j in range(R):
                    i = cc * R + j
                    rsl = spool.tile([P, D], F16, name="rsl", tag="rsl", bufs=2)
                    nc.sync.dma_start(rsl[:], rsout[j * P:(j + 1) * P, :])
                    nc.vector.tensor_add(x_own[i][:], x_own[i][:], rsl[:])

            # ---- initial AllGather for layer-0 attention ----
            for cc in range(NCHUNK):
                ln_ag("a0", cc)

            for l in range(L):
                # ---- attention weights (scalar queue) ----
                wq = wpool.tile([P, KTD * OC], F16, name="wq", tag="wq")
                nc.scalar.dma_start(wq[:], wq_d[l])
                wk = wpool.tile([P, KTD * OC], F16, name="wk", tag="wk")
                nc.scalar.dma_start(wk[:], wk_d[l])
                wv = wpool.tile([P, KTD * OC], F16, name="wv", tag="wv")
                nc.scalar.dma_start(wv[:], wv_d[l])
                wo = wpool.tile([P, NOPT * D], F16, name="wo", tag="wo")
                nc.scalar.dma_start(wo[:], wo_d[l])

                # ---- Q/K (channel-major + RoPE) and V (row-major) ----
                qt = [bigpool.tile([P, S], F16, name=f"q{m}", tag=f"big{m}")
                      for m in range(NOPT)]
                ktt = [bigpool.tile([P, S], F16, name=f"k{m}",
                                    tag=f"big{NOPT + m}")
                       for m in range(NOPT)]
                vt = [vpool.tile([P, LH * 65], F16, name=f"v{rt}", tag=f"v{rt}")
                      for rt in range(NRA)]

                for sc in range(NQC):
                    ns = sc * QC
                    for wsb, outs in ((wq, qt), (wk, ktt)):
                        for m in range(NOPT):
                            pq = ppool.tile([P, QC], F32, name="pq", tag="proj",
                                            bufs=2, space="PSUM")
                            for kt in range(KTD):
                                nc.tensor.matmul(
                                    pq[:],
                                    lhsT=wsb[:, kt * OC + m * P:
                                             kt * OC + (m + 1) * P],
                                    rhs=ht[:, kt, ns:ns + QC],
                                    start=(kt == 0), stop=(kt == KTD - 1))
                            nc.scalar.copy(outs[m][:, ns:ns + QC], pq[:])
                        # RoPE on this column chunk: rot via sign-folded
                        # permutation matmul, combine on DVE.
                        for t in outs:
                            prot = ppool.tile([P, QC], F32, name="prot",
                                              tag="po", bufs=2, space="PSUM")
                            nc.tensor.matmul(prot[:], lhsT=rott[:],
                                             rhs=t[:, ns:ns + QC],
                                             start=True, stop=True)
                            rs = spool.tile([P, QC], F16, name="rs", tag="rs",
                                            bufs=2)
                            nc.vector.tensor_mul(rs[:], prot[:],
                                                 sinb[:, ns:ns + QC])
                            nc.vector.tensor_mul(t[:, ns:ns + QC],
                                                 t[:, ns:ns + QC],
                                                 cosb[:, ns:ns + QC])
                            nc.vector.tensor_add(t[:, ns:ns + QC],
                                                 t[:, ns:ns + QC], rs[:])
                    # V for the row-tiles of this chunk
                    for rt in range(sc * RPC, (sc + 1) * RPC):
                        v = vt[rt]
                        v3 = v[:].rearrange("p (h c) -> p h c", c=65)
                        nc.vector.memset(v3[:, :, 64:65], 1.0)
                        pv = ppool.tile([P, 512], F32, name="pv", tag="proj",
                                        bufs=2, space="PSUM")
                        for kt in range(KTD):
                            nc.tensor.matmul(
                                pv[:, :OC],
                                lhsT=ht[:, kt, rt * P:(rt + 1) * P],
                                rhs=wv[:, kt * OC:(kt + 1) * OC],
                                start=(kt == 0), stop=(kt == KTD - 1))
                        nc.scalar.copy(
                            v3[:, :, :64],
                            pv[:, :OC].rearrange("p (h c) -> p h c", c=64))

                # ---- flash-style causal attention + Wo + RS, per q-chunk ----
                ot = [bigpool.tile([P, S], F16, name=f"o{m}",
                                   tag=f"big{2 * NOPT + m}")
                      for m in range(NOPT)]
                for qc in range(NQC):
                    q0 = qc * QC
                    nkt = (q0 + QC) // P
                    sums_hp = []
                    for hp in range(NOPT):          # head pair (2hp, 2hp+1)
                        h0, h1 = 2 * hp, 2 * hp + 1
                        sums = spool.tile([1, 2 * QC], F32, name="sums",
                                          tag="sums", bufs=2)
                        sums_hp.append(sums)
                        po0 = ppool.tile([P, QC], F32, name="po0", tag="po",
                                         bufs=2, space="PSUM")
                        po1 = ppool.tile([P, QC], F32, name="po1", tag="po",
                                         bufs=2, space="PSUM")
                        for kt in range(nkt):
                            ps = ppool.tile([P, 2, QC], F32, name="ps",
                                            tag="sc", bufs=2, space="PSUM")
                            nc.tensor.matmul(
                                ps[:, 0, :],
                                lhsT=ktt[hp][0:HD, kt * P:(kt + 1) * P],
                                rhs=qt[hp][0:HD, q0:q0 + QC],
                                start=True, stop=True)
                            nc.tensor.matmul(
                                ps[:, 1, :],
                                lhsT=ktt[hp][HD:P, kt * P:(kt + 1) * P],
                                rhs=qt[hp][HD:P, q0:q0 + QC],
                                start=True, stop=True)
                            ex = spool.tile([P, 2, QC], F16, name="ex",
                                            tag="ex", bufs=2)
                            nc.scalar.activation(ex[:], ps[:], AF.Exp,
                                                 bias=zb[:, :1], scale=ISCALE)
                            dd = kt - q0 // P
                            if dd >= 0:
                                nc.vector.tensor_mul(
                                    ex[:], ex[:],
                                    maskt[:, dd * 2 * QC:(dd + 1) * 2 * QC]
                                    .rearrange("p (t q) -> p t q", t=2))
                            nc.tensor.matmul(
                                po0[0:65, :],
                                lhsT=vt[kt][:, h0 * 65:(h0 + 1) * 65],
                                rhs=ex[:, 0, :],
                                start=(kt == 0), stop=(kt == nkt - 1))
                            nc.tensor.matmul(
                                po1[0:65, :],
                                lhsT=vt[kt][:, h1 * 65:(h1 + 1) * 65],
                                rhs=ex[:, 1, :],
                                start=(kt == 0), stop=(kt == nkt - 1))
                        # unnormalized o' and the denominator rows
                        nc.vector.tensor_copy(ot[hp][0:HD, q0:q0 + QC],
                                              po0[0:64, :])
                        nc.vector.tensor_copy(sums[0:1, 0:QC], po0[64:65, :])
                        nc.vector.tensor_copy(ot[hp][HD:P, q0:q0 + QC],
                                              po1[0:64, :])
                        nc.vector.tensor_copy(sums[0:1, QC:2 * QC],
                                              po1[64:65, :])
                        nc.vector.reciprocal_approx_fast(sums[:], sums[:])
                    for h in range(LH):
                        hp, ho = divmod(h * HD, P)
                        pb = ppool.tile([64, QC], F32, name="pb", tag="po",
                                        bufs=2, space="PSUM")
                        nc.tensor.matmul(
                            pb[:], lhsT=ones64[0:1, :],
                            rhs=sums_hp[hp][0:1, (h % 2) * QC:(h % 2 + 1) * QC],
                            start=True, stop=True)
                        nc.vector.tensor_mul(ot[hp][ho:ho + HD, q0:q0 + QC],
                                             ot[hp][ho:ho + HD, q0:q0 + QC],
                                             pb[:])
                    # Wo for this chunk's row-tiles -> RS -> x += ; then the
                    # FFN-phase LN/AllGather for the same chunk.
                    rsin = dpool.tile([CW, D], F16, name=f"rsi_a{l}_{qc}",
                                      tag="rsin", bufs=2 * NCHUNK)
                    for rt in range(qc * RPC, (qc + 1) * RPC):
                        ow = spool.tile([P, D], F16, name="ow", tag="ow",
                                        bufs=2)
                        for (ds, dl) in d_chunks:
                            pw = ppool.tile([P, 512], F32, name="pw",
                                            tag="proj", bufs=2, space="PSUM")
                            for n in range(NOPT):
                                nc.tensor.matmul(
                                    pw[:, :dl],
                                    lhsT=ot[n][:, rt * P:(rt + 1) * P],
                                    rhs=wo[:, n * D + ds: n * D + ds + dl],
                                    start=(n == 0), stop=(n == NOPT - 1))
                            nc.vector.tensor_copy(ow[:, ds:ds + dl],
                                                  pw[:, :dl])
                        nc.sync.dma_start(
                            rsin[(rt - qc * RPC) * P:(rt - qc * RPC + 1) * P, :],
                            ow[:])
                    rs_chunk_add(rsin, f"a{l}", qc)
                    ln_ag(f"f{l}", qc)

                # ---- FFN ----
                wg = wpool.tile([P, KTD * FL], F16, name="wg", tag="wg")
                nc.scalar.dma_start(wg[:], wg_d[l])
                wu = wpool.tile([P, KTD * FL], F16, name="wu", tag="wu")
                nc.scalar.dma_start(wu[:], wu_d[l])
                wd = wpool.tile([P, KTF * D], F16, name="wd", tag="wd")
                nc.scalar.dma_start(wd[:], wd_d[l])

                for sc in range(NQC):
                    ns = sc * QC
                    at = []
                    for fm in range(KTF):
                        a = bigpool.tile([P, QC], F16, name=f"a{fm}",
                                         tag=f"act{fm}", bufs=2)
                        at.append(a)
                        pg = ppool.tile([P, QC], F32, name="pg", tag="proj",
                                        bufs=2, space="PSUM")
                        for kt in range(KTD):
                            nc.tensor.matmul(
                                pg[:],
                                lhsT=wg[:, kt * FL + fm * P:
                                        kt * FL + (fm + 1) * P],
                                rhs=ht[:, kt, ns:ns + QC],
                                start=(kt == 0), stop=(kt == KTD - 1))
                        pu = ppool.tile([P, QC], F32, name="pu", tag="po",
                                        bufs=2, space="PSUM")
                        for kt in range(KTD):
                            nc.tensor.matmul(
                                pu[:],
                                lhsT=wu[:, kt * FL + fm * P:
                                        kt * FL + (fm + 1) * P],
                                rhs=ht[:, kt, ns:ns + QC],
                                start=(kt == 0), stop=(kt == KTD - 1))
                        sg = spool.tile([P, QC], F16, name="sg", tag="sg",
                                        bufs=2)
                        nc.scalar.activation(sg[:], pg[:], AF.Silu,
                                             bias=zb[:, :1])
                        nc.vector.tensor_mul(at[fm][:], sg[:], pu[:])
                    # Wd for this chunk -> RS -> x += ; then next-phase AG
                    rsin2 = dpool.tile([CW, D], F16, name=f"rsi_f{l}_{sc}",
                                       tag="rsin", bufs=2 * NCHUNK)
                    for rt in range(sc * RPC, (sc + 1) * RPC):
                        dw = spool.tile([P, D], F16, name="dw", tag="ow",
                                        bufs=2)
                        for (ds, dl) in d_chunks:
                            pd = ppool.tile([P, 512], F32, name="pd",
                                            tag="proj", bufs=2, space="PSUM")
                            for kt in range(KTF):
                                nc.tensor.matmul(
                                    pd[:, :dl],
                                    lhsT=at[kt][:, (rt - sc * RPC) * P:
                                               (rt - sc * RPC + 1) * P],
                                    rhs=wd[:, kt * D + ds: kt * D + ds + dl],
                                    start=(kt == 0), stop=(kt == KTF - 1))
                            nc.scalar.copy(dw[:, ds:ds + dl], pd[:, :dl])
                        nc.sync.dma_start(
                            rsin2[(rt - sc * RPC) * P:
                                  (rt - sc * RPC + 1) * P, :],
                            dw[:])
                    rs_chunk_add(rsin2, f"f{l}", sc)
                    if l < L - 1:
                        ln_ag(f"a{l + 1}", sc)
                    else:
                        # final RMS norm * fnorm for this chunk's rows
                        for j in range(R):
                            i = sc * R + j
                            fo = spool.tile([P, D], F32, name="fo", tag="fo",
                                            bufs=1)
                            inv = rms_inv(x_own[i][:], fo[:])
                            nc.vector.tensor_scalar_mul(fo[:], x_own[i][:],
                                                        inv[:, :1])
                            nc.vector.tensor_mul(fo[:], fo[:], fnormb[:])
                            nc.sync.dma_start(out_d[i * P:(i + 1) * P, :],
                                              fo[:])

    nc.compile()
    return nc


# --------------------------------------------------------------------------
# host-side input preparation
# --------------------------------------------------------------------------

def _own_rowtiles(c, tp):
    """Global row-tile indices owned by rank tp, in local order."""
    return [(cc * TP + tp) * c["R"] + j
            for cc in range(c["NCHUNK"]) for j in range(c["R"])]


def make_in_maps(cfg, inputs):
    c = _derive(cfg)
    B, S, D, K, H, HD, F, L = (c[k] for k in ("B", "S", "D", "K", "H", "HD", "F", "L"))
    LH, OC, NOPT, OWN = c["LH"], c["OC"], c["NOPT"], c["OWN"]
    KTD, FL, KTF = c["KTD"], c["FL"], c["KTF"]

    gi = {k: np.asarray(v) for k, v in inputs.items() if k != "seq_len"}
    x_processed = gi["x_processed"].astype(np.float32)
    boundaries = gi["boundaries"].astype(np.int64)
    counts = gi["counts"].astype(np.int64)
    x_residual = gi["x_residual"].astype(np.float32)
    cos = gi["cos"].astype(np.float32)
    sin = gi["sin"].astype(np.float32)
    start_emb = gi["start_emb"].astype(np.float32)
    ln1 = gi["ln1"].astype(np.float32)
    ln2 = gi["ln2"].astype(np.float32)
    fnorm = gi["fnorm"].astype(np.float32)
    Wq, Wk, Wv, Wo = (gi[k].astype(np.float32) for k in ("Wq", "Wk", "Wv", "Wo"))
    Wg, Wu, Wd = (gi[k].astype(np.float32) for k in ("Wg", "Wu", "Wd"))

    # segment index per position (searchsorted over masked boundaries)
    seg = np.empty((B, S), np.int32)
    for b in range(B):
        bnd = np.where(np.arange(K) < counts[b], boundaries[b], S)
        seg[b] = np.searchsorted(bnd, np.arange(S), side="left").astype(np.int32)

    pidx = np.arange(P)
    cosb = cos.T[pidx % HD].astype(np.float16)                       # [P, S]
    sinb = sin.T[pidx % HD].astype(np.float16)                       # [P, S]
    # sign-folded rotation permutation: out[m] = -in[m+32] (first half),
    # out[m] = +in[m-32] (second half), per 64-wide head block.
    rott = np.zeros((P, P), np.float16)
    for m in range(P):
        mm = m % HD
        if mm < HD // 2:
            rott[(m // HD) * HD + mm + HD // 2, m] = -1.0
        else:
            rott[(m // HD) * HD + mm - HD // 2, m] = 1.0
    fnormb = np.broadcast_to(fnorm, (P, D)).astype(np.float16).copy()
    QCc, RPCc = c["QC"], c["CW"] // P
    kk = np.arange(P)[:, None]
    qq = np.arange(QCc)[None, :]
    masks = np.concatenate(
        [np.concatenate([(dd * P + kk <= qq).astype(np.float16)] * 2, axis=1)
         for dd in range(RPCc)],
        axis=1)                                                       # [P, RPC*2*QC]

    def kt_layout(w):      # [D or FL, C] -> [P, KT*C]
        n, cdim = w.shape
        return np.ascontiguousarray(
            w.reshape(n // P, P, cdim).transpose(1, 0, 2).reshape(P, -1)
        ).astype(np.float16)

    in_maps = []
    for cid in range(NCORES):
        b, tp = divmod(cid, TP)
        gts = _own_rowtiles(c, tp)
        rows = np.concatenate([np.arange(g * P, (g + 1) * P) for g in gts])
        hb = tp * OC
        fb = tp * FL
        reps = np.concatenate([start_emb[None], x_processed[b]], axis=0)
        m = dict(
            reps=np.ascontiguousarray(reps).astype(np.float16),
            segq=np.ascontiguousarray(seg[b, rows].reshape(OWN, 1)),
            resid=np.ascontiguousarray(x_residual[b, rows]).astype(np.float16),
            cosb=cosb, sinb=sinb, rott=rott, fnormb=fnormb, masks=masks,
            wq=np.stack([kt_layout(ln1[l][:, None] * Wq[l][:, hb:hb + OC])
                         for l in range(L)]),
            wk=np.stack([kt_layout(ln1[l][:, None] * Wk[l][:, hb:hb + OC])
                         for l in range(L)]),
            wv=np.stack([kt_layout(ln1[l][:, None] * Wv[l][:, hb:hb + OC])
                         for l in range(L)]),
            wo=np.stack([kt_layout(Wo[l][hb:hb + OC, :]) for l in range(L)]),
            wg=np.stack([kt_layout(ln2[l][:, None] * Wg[l][:, fb:fb + FL])
                         for l in range(L)]),
            wu=np.stack([kt_layout(ln2[l][:, None] * Wu[l][:, fb:fb + FL])
                         for l in range(L)]),
            wd=np.stack([kt_layout(Wd[l][fb:fb + FL, :]) for l in range(L)]),
        )
        in_maps.append(m)
    return in_maps


def assemble_output(cfg, results):
    c = _derive(cfg)
    B, S, D, OWN = c["B"], c["S"], c["D"], c["OWN"]
    out = np.empty((B, S, D), np.float32)
    for cid in range(NCORES):
        b, tp = divmod(cid, TP)
        gts = _own_rowtiles(c, tp)
        r = results[cid]["out"]
        for i, g in enumerate(gts):
            out[b, g * P:(g + 1) * P] = r[i * P:(i + 1) * P]
    return out


_NC_CACHE = {}


def _get_nc(cfg):
    key = tuple(sorted(cfg.items()))
    if key not in _NC_CACHE:
        _NC_CACHE[key] = build_nc(cfg)
    return _NC_CACHE[key]


def kernel(**inputs) -> np.ndarray:
    cfg = FULL_CFG
    nc = _get_nc(cfg)
    in_maps = make_in_maps(cfg, inputs)
    res = bass_utils.run_bass_kernel_spmd(nc, in_maps,
                                          core_ids=list(range(NCORES)))
    return assemble_output(cfg, res.results)
